# revision 10
# baseline (speedup 1.0000x reference)
"""Trainium2 Bass kernel for nn_AttLayer (4-head attention, softmax over queries).

Sharding: data-parallel over batch. 8 batch elements -> 8 NeuronCores, zero
collectives. Each core computes one batch element entirely in "transposed"
layout (channels/features on partitions, sequence on the free dim), which
makes every DMA layout-natural:

  x_b            : [64, 1024]  (natural layout of x[b] = [64, 32, 32])
  projT          : [7680, 1024] = W_aug^T @ x_aug   (bias via ones-row trick)
  scoresT[j, i]  : kT^T(d-tiles) @ qT               (j on partitions)
  softmax over i : free-dim reduction; exp+rowsum fused in one ACT op
  attT[d, i]     : v^T(j-tiles) @ (256*exp/den)     (normalized weights in fp8)
  outT [64,1024] : W_out^T @ attT + b_out + x_b     (accumulated in PSUM)

q/k/v and the normalized attention weights are fp8e4 so the two big matmuls
(scores, att) run in DoubleRow perf mode; the out-projection stays bf16.
The normalized weights 256*exp[j,i]/den[j] are bounded by 256 (each exp term
is a summand of its own denominator), so fp8e4 never overflows.
"""

import numpy as np
import ml_dtypes

import concourse.bass as bass
import concourse.tile as tile
from concourse import bacc, mybir
from concourse.bass_utils import run_bass_kernel_spmd

NH = 4          # heads
D = 640         # per-head dim
C = 64          # channels
SEQ = 1024      # 32*32
SCALE = float(D) ** -0.5
N_CORES = 8
FP = mybir.dt.float32
BF = mybir.dt.bfloat16
F8 = mybir.dt.float8e4
ES_SCALE = 256.0  # keep normalized weights inside fp8e4 normal range

JT = SEQ // 128     # 8 j-tiles (128 keys each)
DT = D // 128       # 5 d-tiles per head
IC = SEQ // 512     # 2 i-chunks (512 queries each)
KT_OUT = NH * D // 128  # 20 contraction tiles for the output projection
HPA3 = 3 * NH * D   # 7680

AF = mybir.ActivationFunctionType
ALU = mybir.AluOpType
DR = mybir.MatmulPerfMode.DoubleRow


def _build():
    nc = bacc.Bacc(None, target_bir_lowering=False)
    xa = nc.declare_dram_parameter("xa", [C + 1, SEQ], BF, isOutput=False)
    xf = nc.declare_dram_parameter("xf", [C, SEQ], FP, isOutput=False)
    wa = nc.declare_dram_parameter("wa", [C + 1, HPA3], BF, isOutput=False)
    wo = nc.declare_dram_parameter("wo", [128, KT_OUT, C], BF, isOutput=False)
    bo = nc.declare_dram_parameter("bo", [C, 1], FP, isOutput=False)
    out = nc.declare_dram_parameter("out", [C, SEQ], FP, isOutput=True)

    with tile.TileContext(nc) as tc:
        with (
            tc.tile_pool(name="consts", bufs=1) as consts,
            tc.tile_pool(name="hpool", bufs=2) as hpool,
            tc.tile_pool(name="pr", bufs=3, space="PSUM") as pr_psum,
            tc.tile_pool(name="sc", bufs=3, space="PSUM") as sc_psum,
            tc.tile_pool(name="o2", bufs=1, space="PSUM") as o2_psum,
        ):
            xa_sb = consts.tile([C + 1, SEQ], BF)
            nc.sync.dma_start(out=xa_sb[:], in_=xa[:, :])
            xf_sb = consts.tile([C, SEQ], FP)
            nc.sync.dma_start(out=xf_sb[:], in_=xf[:, :])
            wa_sb = consts.tile([C + 1, HPA3], BF)
            for h in range(NH):  # per-head chunks so head 0 can start early
                nc.sync.dma_start(
                    out=wa_sb[:, h * 3 * D:(h + 1) * 3 * D],
                    in_=wa[:, h * 3 * D:(h + 1) * 3 * D],
                )
            wo_sb = consts.tile([128, KT_OUT, C], BF)
            nc.sync.dma_start(out=wo_sb[:], in_=wo[:, :, :])
            bo_sb = consts.tile([C, 1], FP)
            nc.sync.dma_start(out=bo_sb[:], in_=bo[:, :])
            out_sb = consts.tile([C, SEQ], FP)

            # output-projection accumulators, live across all heads
            o2_tiles = [
                o2_psum.tile([C, 512], FP, tag=f"o2_{ic}", name=f"o2_{ic}")
                for ic in range(IC)
            ]

            for h in range(NH):
                qcol = h * 3 * D
                kcol = qcol + D
                vcol = qcol + 2 * D

                qT = hpool.tile([128, DT, SEQ], F8, tag="qT")
                kT = hpool.tile([128, DT, SEQ], F8, tag="kT")
                vs = hpool.tile([128, JT, D], F8, tag="vs")
                esr = hpool.tile([128, JT, SEQ], BF, tag="esr")   # raw exp
                es = hpool.tile([128, JT, SEQ], F8, tag="es")     # 256*exp/den
                aT = hpool.tile([128, DT, SEQ], BF, tag="aT")
                dpart = hpool.tile([128, JT, IC], FP, tag="dpart")
                den = hpool.tile([128, JT], FP, tag="den")
                rec = hpool.tile([128, JT], FP, tag="rec")

                # ---- q^T / k^T projections: psum[wcol-tile, i] = wa^T @ xa
                for col0, dst, eng in ((qcol, qT, "s"), (kcol, kT, "v")):
                    for d0 in range(DT):
                        for ic in range(IC):
                            ps = pr_psum.tile([128, 512], FP, tag="pr")
                            nc.tensor.matmul(
                                ps[:],
                                lhsT=wa_sb[:, col0 + d0 * 128: col0 + (d0 + 1) * 128],
                                rhs=xa_sb[:, ic * 512:(ic + 1) * 512],
                                start=True, stop=True,
                            )
                            dst_ap = dst[:, d0, ic * 512:(ic + 1) * 512]
                            if eng == "v":
                                nc.vector.tensor_copy(out=dst_ap, in_=ps[:])
                            else:
                                nc.scalar.copy(out=dst_ap, in_=ps[:])

                # ---- v projection (plain fp8 copy; 1/den now lives in es)
                for jt in range(JT):
                    for vc0, vcw in ((0, 512), (512, 128)):
                        ps = pr_psum.tile([128, 512], FP, tag="pr")
                        nc.tensor.matmul(
                            ps[:, :vcw],
                            lhsT=xa_sb[:, jt * 128:(jt + 1) * 128],
                            rhs=wa_sb[:, vcol + vc0: vcol + vc0 + vcw],
                            start=True, stop=True,
                        )
                        nc.scalar.copy(out=vs[:, jt, vc0:vc0 + vcw], in_=ps[:, :vcw])

                # ---- scoresT + fused exp/rowsum, then per-j-tile normalize
                for jt in range(JT):
                    pss = [
                        sc_psum.tile([128, 512], FP, tag="sc", name=f"sc_{jt}_{ic}")
                        for ic in range(IC)
                    ]
                    for kk in (0, 2, 4):  # lhsT shared across the ic pair
                        for ic in range(IC):
                            if kk < 4:
                                nc.tensor.matmul(
                                    pss[ic][:],
                                    lhsT=kT[:, kk:kk + 2, jt * 128:(jt + 1) * 128],
                                    rhs=qT[:, kk:kk + 2, ic * 512:(ic + 1) * 512],
                                    start=(kk == 0), stop=False,
                                    perf_mode=DR,
                                )
                            else:
                                nc.tensor.matmul(
                                    pss[ic][:],
                                    lhsT=kT[:, 4, jt * 128:(jt + 1) * 128],
                                    rhs=qT[:, 4, ic * 512:(ic + 1) * 512],
                                    start=False, stop=True,
                                )
                    for ic in range(IC):
                        nc.scalar.activation(
                            out=esr[:, jt, ic * 512:(ic + 1) * 512],
                            in_=pss[ic][:],
                            func=AF.Exp,
                            scale=SCALE,
                            accum_out=dpart[:, jt, ic:ic + 1],
                        )
                    # den_jt = (sum_i exp)/ES_SCALE; es = exp/den_jt in fp8
                    nc.vector.tensor_add(
                        out=den[:, jt:jt + 1],
                        in0=dpart[:, jt, 0:1], in1=dpart[:, jt, 1:2],
                    )
                    nc.vector.tensor_scalar_mul(
                        den[:, jt:jt + 1], den[:, jt:jt + 1], 1.0 / ES_SCALE,
                    )
                    nc.vector.reciprocal(out=rec[:, jt:jt + 1], in_=den[:, jt:jt + 1])
                    nc.gpsimd.tensor_scalar(
                        out=es[:, jt, :],
                        in0=esr[:, jt, :],
                        scalar1=rec[:, jt:jt + 1],
                        scalar2=None,
                        op0=ALU.mult,
                    )

                # ---- attT[d, i] = sum_j v[j, d] * es[j, i], undo ES_SCALE
                for d0 in range(DT):
                    psa = [
                        pr_psum.tile([128, 512], FP, tag="pr", name=f"at_{d0}_{ic}")
                        for ic in range(IC)
                    ]
                    for jp in range(0, JT, 2):  # lhsT shared across the ic pair
                        for ic in range(IC):
                            nc.tensor.matmul(
                                psa[ic][:],
                                lhsT=vs[:, jp:jp + 2, d0 * 128:(d0 + 1) * 128],
                                rhs=es[:, jp:jp + 2, ic * 512:(ic + 1) * 512],
                                start=(jp == 0), stop=(jp == JT - 2),
                                perf_mode=DR,
                            )
                    for ic in range(IC):
                        nc.vector.tensor_scalar_mul(
                            aT[:, d0, ic * 512:(ic + 1) * 512], psa[ic][:], 1.0 / ES_SCALE,
                        )

                # ---- output projection: out2T += W_out^T @ attT
                for d0 in range(DT):
                    kt = h * DT + d0
                    for ic in range(IC):
                        nc.tensor.matmul(
                            o2_tiles[ic][:],
                            lhsT=wo_sb[:, kt, :],
                            rhs=aT[:, d0, ic * 512:(ic + 1) * 512],
                            start=(kt == 0), stop=(kt == KT_OUT - 1),
                            skip_group_check=True,
                        )

            # ---- epilogue: + b_out (per-partition) + residual x, DMA out
            for ic in range(IC):
                nc.scalar.activation(
                    out=out_sb[:, ic * 512:(ic + 1) * 512],
                    in_=o2_tiles[ic][:],
                    func=AF.Identity,
                    bias=bo_sb[:, 0:1],
                    scale=1.0,
                )
            nc.vector.tensor_add(out=out_sb[:], in0=out_sb[:], in1=xf_sb[:])
            nc.sync.dma_start(out=out[:, :], in_=out_sb[:])

    nc.compile()
    return nc


_CACHE: dict = {}


def _get_nc():
    if "nc" not in _CACHE:
        _CACHE["nc"] = _build()
    return _CACHE["nc"]


def _prep_in_maps(x, W_proj, b_proj, W_out, b_out):
    bf = ml_dtypes.bfloat16
    x = np.ascontiguousarray(np.asarray(x, dtype=np.float32))
    x2 = x.reshape(N_CORES, C, SEQ)
    xa_all = np.empty((N_CORES, C + 1, SEQ), dtype=bf)
    xa_all[:, :C, :] = x2.astype(bf)
    xa_all[:, C, :] = np.float32(1.0)

    wa = np.empty((C + 1, HPA3), dtype=bf)
    wa[:C] = np.asarray(W_proj, dtype=np.float32).astype(bf)
    wa[C] = np.asarray(b_proj, dtype=np.float32).astype(bf)

    wo = np.ascontiguousarray(
        np.asarray(W_out, dtype=np.float32).reshape(KT_OUT, 128, C)
        .transpose(1, 0, 2).astype(bf)
    )
    bo = np.ascontiguousarray(np.asarray(b_out, dtype=np.float32).reshape(C, 1))

    return [
        {
            "xa": np.ascontiguousarray(xa_all[i]),
            "xf": np.ascontiguousarray(x2[i]),
            "wa": wa,
            "wo": wo,
            "bo": bo,
        }
        for i in range(N_CORES)
    ]


def run(x, t, W_proj, b_proj, W_out, b_out, trace=False, **trace_kwargs):
    in_maps = _prep_in_maps(x, W_proj, b_proj, W_out, b_out)
    res = run_bass_kernel_spmd(
        _get_nc(), in_maps, core_ids=list(range(N_CORES)),
        trace=trace, **trace_kwargs,
    )
    out = np.stack([res.results[i]["out"] for i in range(N_CORES)])
    return out.reshape(N_CORES, C, 32, 32), res


def kernel(x, t=None, W_proj=None, b_proj=None, W_out=None, b_out=None):
    out, _ = run(x, t, W_proj, b_proj, W_out, b_out, trace=False)
    return out


# revision 11
# speedup vs baseline: 2.9844x; 2.9844x over previous
"""Trainium2 Bass kernel for nn_AttLayer (4-head attention, softmax over queries).

Sharding: data-parallel over batch. 8 batch elements -> 8 NeuronCores, zero
collectives. Each core computes one batch element entirely in "transposed"
layout (channels/features on partitions, sequence on the free dim), which
makes every DMA layout-natural:

  x_b            : [64, 1024]  (natural layout of x[b] = [64, 32, 32])
  projT          : [7680, 1024] = W_aug^T @ x_aug   (bias via ones-row trick)
  scoresT[j, i]  : kT^T(d-tiles) @ qT               (j on partitions)
  softmax over i : free-dim reduction; exp+rowsum fused in one ACT op
  attT[d, i]     : v^T(j-tiles) @ (256*exp/den)     (normalized weights in fp8)
  outT [64,1024] : W_out^T @ attT + b_out + x_b     (accumulated in PSUM)

q/k/v and the normalized attention weights are fp8e4 so the two big matmuls
(scores, att) run in DoubleRow perf mode; the out-projection stays bf16.
The normalized weights 256*exp[j,i]/den[j] are bounded by 256 (each exp term
is a summand of its own denominator), so fp8e4 never overflows.
"""

import numpy as np
import ml_dtypes

import concourse.bass as bass
import concourse.tile as tile
from concourse import bacc, mybir
from concourse.bass_utils import run_bass_kernel_spmd

NH = 4          # heads
D = 640         # per-head dim
C = 64          # channels
SEQ = 1024      # 32*32
SCALE = float(D) ** -0.5
N_CORES = 8
FP = mybir.dt.float32
BF = mybir.dt.bfloat16
F8 = mybir.dt.float8e4
ES_SCALE = 256.0  # keep normalized weights inside fp8e4 normal range

JT = SEQ // 128     # 8 j-tiles (128 keys each)
DT = D // 128       # 5 d-tiles per head
IC = SEQ // 512     # 2 i-chunks (512 queries each)
KT_OUT = NH * D // 128  # 20 contraction tiles for the output projection
HPA3 = 3 * NH * D   # 7680

AF = mybir.ActivationFunctionType
ALU = mybir.AluOpType
DR = mybir.MatmulPerfMode.DoubleRow


def _build():
    nc = bacc.Bacc(None, target_bir_lowering=False)
    xa = nc.declare_dram_parameter("xa", [C + 1, SEQ], BF, isOutput=False)
    xf = nc.declare_dram_parameter("xf", [C, SEQ], FP, isOutput=False)
    wa = nc.declare_dram_parameter("wa", [C + 1, HPA3], BF, isOutput=False)
    wo = nc.declare_dram_parameter("wo", [128, KT_OUT, C], BF, isOutput=False)
    bo = nc.declare_dram_parameter("bo", [C, 1], FP, isOutput=False)
    out = nc.declare_dram_parameter("out", [C, SEQ], FP, isOutput=True)

    with tile.TileContext(nc) as tc:
        with (
            tc.tile_pool(name="consts", bufs=1) as consts,
            tc.tile_pool(name="hpool", bufs=2) as hpool,
            tc.tile_pool(name="pr", bufs=3, space="PSUM") as pr_psum,
            tc.tile_pool(name="sc", bufs=3, space="PSUM") as sc_psum,
            tc.tile_pool(name="o2", bufs=1, space="PSUM") as o2_psum,
        ):
            xa_sb = consts.tile([C + 1, SEQ], BF)
            nc.sync.dma_start(out=xa_sb[:], in_=xa[:, :])
            xf_sb = consts.tile([C, SEQ], FP)
            nc.sync.dma_start(out=xf_sb[:], in_=xf[:, :])
            wa_sb = consts.tile([C + 1, HPA3], BF)
            for h in range(NH):  # per-head chunks so head 0 can start early
                nc.sync.dma_start(
                    out=wa_sb[:, h * 3 * D:(h + 1) * 3 * D],
                    in_=wa[:, h * 3 * D:(h + 1) * 3 * D],
                )
            wo_sb = consts.tile([128, KT_OUT, C], BF)
            nc.sync.dma_start(out=wo_sb[:], in_=wo[:, :, :])
            bo_sb = consts.tile([C, 1], FP)
            nc.sync.dma_start(out=bo_sb[:], in_=bo[:, :])
            out_sb = consts.tile([C, SEQ], FP)

            # output-projection accumulators, live across all heads
            o2_tiles = [
                o2_psum.tile([C, 512], FP, tag=f"o2_{ic}", name=f"o2_{ic}")
                for ic in range(IC)
            ]

            for h in range(NH):
                qcol = h * 3 * D
                kcol = qcol + D
                vcol = qcol + 2 * D

                qT = hpool.tile([128, DT, SEQ], F8, tag="qT")
                kT = hpool.tile([128, DT, SEQ], F8, tag="kT")
                vs = hpool.tile([128, JT, D], F8, tag="vs")
                esr = hpool.tile([128, JT, SEQ], BF, tag="esr")   # raw exp
                es = hpool.tile([128, JT, SEQ], F8, tag="es")     # 256*exp/den
                aT = hpool.tile([128, DT, SEQ], BF, tag="aT")
                dpart = hpool.tile([128, JT, IC], FP, tag="dpart")
                den = hpool.tile([128, JT], FP, tag="den")
                rec = hpool.tile([128, JT], FP, tag="rec")

                # ---- q^T / k^T projections: psum[wcol-tile, i] = wa^T @ xa
                for col0, dst, eng in ((qcol, qT, "s"), (kcol, kT, "v")):
                    for d0 in range(DT):
                        for ic in range(IC):
                            ps = pr_psum.tile([128, 512], FP, tag="pr")
                            nc.tensor.matmul(
                                ps[:],
                                lhsT=wa_sb[:, col0 + d0 * 128: col0 + (d0 + 1) * 128],
                                rhs=xa_sb[:, ic * 512:(ic + 1) * 512],
                                start=True, stop=True,
                            )
                            dst_ap = dst[:, d0, ic * 512:(ic + 1) * 512]
                            if eng == "v":
                                nc.vector.tensor_copy(out=dst_ap, in_=ps[:])
                            else:
                                nc.scalar.copy(out=dst_ap, in_=ps[:])

                # ---- v projection (plain fp8 copy; 1/den now lives in es)
                for jt in range(JT):
                    for vc0, vcw in ((0, 512), (512, 128)):
                        ps = pr_psum.tile([128, 512], FP, tag="pr")
                        nc.tensor.matmul(
                            ps[:, :vcw],
                            lhsT=xa_sb[:, jt * 128:(jt + 1) * 128],
                            rhs=wa_sb[:, vcol + vc0: vcol + vc0 + vcw],
                            start=True, stop=True,
                        )
                        nc.scalar.copy(out=vs[:, jt, vc0:vc0 + vcw], in_=ps[:, :vcw])

                # ---- scoresT + fused exp/rowsum, then per-j-tile normalize
                for jt in range(JT):
                    pss = [
                        sc_psum.tile([128, 512], FP, tag="sc", name=f"sc_{jt}_{ic}")
                        for ic in range(IC)
                    ]
                    for kk in (0, 2, 4):  # lhsT shared across the ic pair
                        for ic in range(IC):
                            if kk < 4:
                                nc.tensor.matmul(
                                    pss[ic][:],
                                    lhsT=kT[:, kk:kk + 2, jt * 128:(jt + 1) * 128],
                                    rhs=qT[:, kk:kk + 2, ic * 512:(ic + 1) * 512],
                                    start=(kk == 0), stop=False,
                                    perf_mode=DR,
                                )
                            else:
                                nc.tensor.matmul(
                                    pss[ic][:],
                                    lhsT=kT[:, 4, jt * 128:(jt + 1) * 128],
                                    rhs=qT[:, 4, ic * 512:(ic + 1) * 512],
                                    start=False, stop=True,
                                )
                    for ic in range(IC):
                        nc.scalar.activation(
                            out=esr[:, jt, ic * 512:(ic + 1) * 512],
                            in_=pss[ic][:],
                            func=AF.Exp,
                            scale=SCALE,
                            accum_out=dpart[:, jt, ic:ic + 1],
                        )
                    # den_jt = (sum_i exp)/ES_SCALE; es = exp/den_jt in fp8
                    nc.vector.tensor_add(
                        out=den[:, jt:jt + 1],
                        in0=dpart[:, jt, 0:1], in1=dpart[:, jt, 1:2],
                    )
                    nc.vector.tensor_scalar_mul(
                        den[:, jt:jt + 1], den[:, jt:jt + 1], 1.0 / ES_SCALE,
                    )
                    nc.vector.reciprocal(out=rec[:, jt:jt + 1], in_=den[:, jt:jt + 1])
                    nc.vector.tensor_scalar(
                        out=es[:, jt, :],
                        in0=esr[:, jt, :],
                        scalar1=rec[:, jt:jt + 1],
                        scalar2=None,
                        op0=ALU.mult,
                    )

                # ---- attT[d, i] = sum_j v[j, d] * es[j, i], undo ES_SCALE
                for d0 in range(DT):
                    psa = [
                        pr_psum.tile([128, 512], FP, tag="pr", name=f"at_{d0}_{ic}")
                        for ic in range(IC)
                    ]
                    for jp in range(0, JT, 2):  # lhsT shared across the ic pair
                        for ic in range(IC):
                            nc.tensor.matmul(
                                psa[ic][:],
                                lhsT=vs[:, jp:jp + 2, d0 * 128:(d0 + 1) * 128],
                                rhs=es[:, jp:jp + 2, ic * 512:(ic + 1) * 512],
                                start=(jp == 0), stop=(jp == JT - 2),
                                perf_mode=DR,
                            )
                    for ic in range(IC):
                        nc.vector.tensor_scalar_mul(
                            aT[:, d0, ic * 512:(ic + 1) * 512], psa[ic][:], 1.0 / ES_SCALE,
                        )

                # ---- output projection: out2T += W_out^T @ attT
                for d0 in range(DT):
                    kt = h * DT + d0
                    for ic in range(IC):
                        nc.tensor.matmul(
                            o2_tiles[ic][:],
                            lhsT=wo_sb[:, kt, :],
                            rhs=aT[:, d0, ic * 512:(ic + 1) * 512],
                            start=(kt == 0), stop=(kt == KT_OUT - 1),
                            skip_group_check=True,
                        )

            # ---- epilogue: + b_out (per-partition) + residual x, DMA out
            for ic in range(IC):
                nc.scalar.activation(
                    out=out_sb[:, ic * 512:(ic + 1) * 512],
                    in_=o2_tiles[ic][:],
                    func=AF.Identity,
                    bias=bo_sb[:, 0:1],
                    scale=1.0,
                )
            nc.vector.tensor_add(out=out_sb[:], in0=out_sb[:], in1=xf_sb[:])
            nc.sync.dma_start(out=out[:, :], in_=out_sb[:])

    nc.compile()
    return nc


_CACHE: dict = {}


def _get_nc():
    if "nc" not in _CACHE:
        _CACHE["nc"] = _build()
    return _CACHE["nc"]


def _prep_in_maps(x, W_proj, b_proj, W_out, b_out):
    bf = ml_dtypes.bfloat16
    x = np.ascontiguousarray(np.asarray(x, dtype=np.float32))
    x2 = x.reshape(N_CORES, C, SEQ)
    xa_all = np.empty((N_CORES, C + 1, SEQ), dtype=bf)
    xa_all[:, :C, :] = x2.astype(bf)
    xa_all[:, C, :] = np.float32(1.0)

    wa = np.empty((C + 1, HPA3), dtype=bf)
    wa[:C] = np.asarray(W_proj, dtype=np.float32).astype(bf)
    wa[C] = np.asarray(b_proj, dtype=np.float32).astype(bf)

    wo = np.ascontiguousarray(
        np.asarray(W_out, dtype=np.float32).reshape(KT_OUT, 128, C)
        .transpose(1, 0, 2).astype(bf)
    )
    bo = np.ascontiguousarray(np.asarray(b_out, dtype=np.float32).reshape(C, 1))

    return [
        {
            "xa": np.ascontiguousarray(xa_all[i]),
            "xf": np.ascontiguousarray(x2[i]),
            "wa": wa,
            "wo": wo,
            "bo": bo,
        }
        for i in range(N_CORES)
    ]


def run(x, t, W_proj, b_proj, W_out, b_out, trace=False, **trace_kwargs):
    in_maps = _prep_in_maps(x, W_proj, b_proj, W_out, b_out)
    res = run_bass_kernel_spmd(
        _get_nc(), in_maps, core_ids=list(range(N_CORES)),
        trace=trace, **trace_kwargs,
    )
    out = np.stack([res.results[i]["out"] for i in range(N_CORES)])
    return out.reshape(N_CORES, C, 32, 32), res


def kernel(x, t=None, W_proj=None, b_proj=None, W_out=None, b_out=None):
    out, _ = run(x, t, W_proj, b_proj, W_out, b_out, trace=False)
    return out


# revision 17
# speedup vs baseline: 9.2910x; 3.1132x over previous
"""Trainium2 Bass kernel for nn_AttLayer (4-head attention, softmax over queries).

Sharding: data-parallel over batch. 8 batch elements -> 8 NeuronCores, zero
collectives.

Key algebraic restructuring: with C=64 channels the attention is rank-65.
Folding the projections through the score/value contractions (bias rows
appended via the augmented-ones trick):

  R_h       = G_h^T-contracted input               G_h = Wk_aug_h @ Wq_aug_h^T
  scoresT_h = Xa^T R_h                             (= Xa^T G_h Xa, 65 x 65 G)
  es        = exp(SCALE * scoresT)                 row-sum den fused into the
                                                   exp activation (accum_out)
  xtr[j,c]  = XaT[j,c] / den[j]                    reciprocal folded into the
                                                   65-wide transposed input
  M2_h[c,i] = sum_j xtr[j,c] * es[j,i]             (65 x 1024)
  out2     += F_h^T @ M2_h                         F_h = Wv_aug_h @ Wout_h
  out       = out2 + b_out + x

G_h and F_h are computed on the host in f32 (exact). Everything on-chip is
bf16 matmuls with f32 PSUM accumulation; the exp/normalize core is the
critical path (ScalarEngine), so all other work is software-pipelined into
the per-j-tile chain steps of neighboring heads.
"""

import numpy as np
import ml_dtypes

import concourse.bass as bass
import concourse.tile as tile
from concourse import bacc, mybir
from concourse.bass_utils import run_bass_kernel_spmd

NH = 4          # heads
D = 640         # per-head dim
C = 64          # channels
CA = C + 1      # augmented (ones row)
SEQ = 1024      # 32*32
SCALE = float(D) ** -0.5
N_CORES = 8
FP = mybir.dt.float32
BF = mybir.dt.bfloat16

JT = SEQ // 128     # 8 j-tiles (128 keys each)
IC = SEQ // 512     # 2 i-chunks (512 queries each)

AF = mybir.ActivationFunctionType
ALU = mybir.AluOpType


def _build():
    nc = bacc.Bacc(None, target_bir_lowering=False)
    xa = nc.declare_dram_parameter("xa", [CA, SEQ], BF, isOutput=False)
    xt = nc.declare_dram_parameter("xt", [128, JT, CA], BF, isOutput=False)
    xf = nc.declare_dram_parameter("xf", [C, SEQ], FP, isOutput=False)
    gt = nc.declare_dram_parameter("gt", [CA, NH, CA], BF, isOutput=False)
    ff = nc.declare_dram_parameter("ff", [CA, NH, C], BF, isOutput=False)
    bo = nc.declare_dram_parameter("bo", [C, 1], FP, isOutput=False)
    out = nc.declare_dram_parameter("out", [C, SEQ], FP, isOutput=True)

    with tile.TileContext(nc) as tc:
        with (
            tc.tile_pool(name="consts", bufs=1) as consts,
            tc.tile_pool(name="hpool", bufs=2) as hpool,
            tc.tile_pool(name="sc", bufs=2, space="PSUM") as sc_psum,
            tc.tile_pool(name="pm", bufs=2, space="PSUM") as pm_psum,
        ):
            xa_sb = consts.tile([CA, SEQ], BF)
            for ic in range(IC):
                nc.sync.dma_start(
                    out=xa_sb[:, ic * 512:(ic + 1) * 512],
                    in_=xa[:, ic * 512:(ic + 1) * 512],
                )
            gt_sb = consts.tile([CA, NH, CA], BF)
            nc.sync.dma_start(out=gt_sb[:], in_=gt[:, :, :])
            xt_sb = consts.tile([128, JT, CA], BF)
            nc.sync.dma_start(out=xt_sb[:], in_=xt[:, :, :])
            ff_sb = consts.tile([CA, NH, C], BF)
            nc.sync.dma_start(out=ff_sb[:], in_=ff[:, :, :])
            xf_sb = consts.tile([C, SEQ], FP)
            nc.sync.dma_start(out=xf_sb[:], in_=xf[:, :])
            bo_sb = consts.tile([C, 1], FP)
            nc.sync.dma_start(out=bo_sb[:], in_=bo[:, :])
            out_sb = consts.tile([C, SEQ], FP)
            o2acc = consts.tile([C, SEQ], FP)

            def emit_R(h):
                R_sb = hpool.tile([CA, SEQ], BF, tag="R", name=f"R_{h}")
                rps = pm_psum.tile([CA, SEQ], FP, tag="pm", name=f"rp_{h}")
                for ic in range(IC):
                    nc.tensor.matmul(
                        rps[:, ic * 512:(ic + 1) * 512],
                        lhsT=gt_sb[:, h, :],
                        rhs=xa_sb[:, ic * 512:(ic + 1) * 512],
                        start=True, stop=True,
                    )
                for ic in range(IC):
                    nc.vector.tensor_copy(
                        out=R_sb[:, ic * 512:(ic + 1) * 512],
                        in_=rps[:, ic * 512:(ic + 1) * 512],
                    )
                return R_sb

            def emit_M2_mms(mps, xtr, es, jt):
                for ic in range(IC):
                    nc.tensor.matmul(
                        mps[:, ic * 512:(ic + 1) * 512],
                        lhsT=xtr[:, jt, :],
                        rhs=es[:, jt, ic * 512:(ic + 1) * 512],
                        start=(jt == 0), stop=(jt == JT - 1),
                    )

            def emit_m2_conv(ph, pmps):
                pm2 = hpool.tile([CA, SEQ], BF, tag="m2", name=f"m2_{ph}")
                for ic in range(IC):
                    nc.vector.tensor_copy(
                        out=pm2[:, ic * 512:(ic + 1) * 512],
                        in_=pmps[:, ic * 512:(ic + 1) * 512],
                    )
                return pm2

            def emit_out2(h, m2):
                o2p = pm_psum.tile([CA, SEQ], FP, tag="pm", name=f"o2_{h}")
                for ic in range(IC):
                    nc.tensor.matmul(
                        o2p[:C, ic * 512:(ic + 1) * 512],
                        lhsT=ff_sb[:, h, :],
                        rhs=m2[:, ic * 512:(ic + 1) * 512],
                        start=True, stop=True,
                    )
                if h == 0:
                    nc.vector.tensor_copy(out=o2acc[:], in_=o2p[:C, :])
                elif h < NH - 1:
                    nc.vector.tensor_add(out=o2acc[:], in0=o2acc[:], in1=o2p[:C, :])
                else:
                    # final head: o2acc already holds heads 0-2 plus residual
                    for ic in range(IC):
                        sl = slice(ic * 512, (ic + 1) * 512)
                        nc.scalar.activation(
                            out=out_sb[:, sl],
                            in_=o2p[:C, sl],
                            func=AF.Identity,
                            bias=bo_sb[:, 0:1],
                            scale=1.0,
                        )
                        nc.vector.tensor_add(
                            out=out_sb[:, sl], in0=out_sb[:, sl], in1=o2acc[:, sl],
                        )
                        for q in range(2):
                            qsl = slice(ic * 512 + q * 256, ic * 512 + (q + 1) * 256)
                            nc.sync.dma_start(out=out[:, qsl], in_=out_sb[:, qsl])

            R_cur = emit_R(0)
            R_nxt = None
            prev = None   # (h, es, xtr, mps) of the previous head
            for h in range(NH):
                R_sb = R_cur
                last = h == NH - 1
                es = hpool.tile([128, JT, SEQ], BF, tag="es", name=f"es_{h}")
                xtr = hpool.tile([128, JT, CA], BF, tag="xtr", name=f"xtr_{h}")
                den = hpool.tile([128, JT], FP, tag="den", name=f"den_{h}")
                rec = hpool.tile([128, JT], FP, tag="rec", name=f"rec_{h}")
                own_mps = (
                    pm_psum.tile([CA, SEQ], FP, tag="pm", name="mp_last")
                    if last else None
                )

                for jt in range(JT):
                    pst = sc_psum.tile([128, SEQ], FP, tag="sc", name=f"sc_{h}_{jt}")
                    for ic in range(IC):
                        nc.tensor.matmul(
                            pst[:, ic * 512:(ic + 1) * 512],
                            lhsT=xa_sb[:, jt * 128:(jt + 1) * 128],
                            rhs=R_sb[:, ic * 512:(ic + 1) * 512],
                            start=True, stop=True,
                        )
                    nc.scalar.activation(
                        out=es[:, jt, :],
                        in_=pst[:],
                        func=AF.Exp,
                        scale=SCALE,
                        accum_out=den[:, jt:jt + 1],
                    )
                    nc.vector.reciprocal(out=rec[:, jt:jt + 1], in_=den[:, jt:jt + 1])
                    nc.vector.tensor_scalar_mul(
                        xtr[:, jt, :], xt_sb[:, jt, :], rec[:, jt:jt + 1],
                    )

                    # ---- pipelined injections (<=2 matmuls per chain step)
                    if prev is not None:
                        ph, pes, pxtr, pmps = prev
                        emit_M2_mms(pmps, pxtr, pes, jt)
                        if jt == JT - 1:
                            pm2 = emit_m2_conv(ph, pmps)
                            emit_out2(ph, pm2)
                            prev = None
                    if jt == 2 and h + 1 < NH:
                        R_nxt = emit_R(h + 1)
                    if last and jt >= 2:
                        emit_M2_mms(own_mps, xtr, es, jt - 2)
                    if last and jt == 7:
                        nc.vector.tensor_add(
                            out=o2acc[:], in0=o2acc[:], in1=xf_sb[:],
                        )

                if not last:
                    mps = pm_psum.tile([CA, SEQ], FP, tag="pm", name=f"mp_{h}")
                    prev = (h, es, xtr, mps)
                R_cur = R_nxt

            # drain the last head's M2 tail (j-tiles 6, 7) and final output
            es_l, xtr_l = es, xtr
            for jt in (6, 7):
                emit_M2_mms(own_mps, xtr_l, es_l, jt)
            pm2 = emit_m2_conv(NH - 1, own_mps)
            emit_out2(NH - 1, pm2)

    nc.compile()
    return nc


_CACHE: dict = {}


def _get_nc():
    if "nc" not in _CACHE:
        _CACHE["nc"] = _build()
    return _CACHE["nc"]


def _prep_in_maps(x, W_proj, b_proj, W_out, b_out):
    bf = ml_dtypes.bfloat16
    x = np.ascontiguousarray(np.asarray(x, dtype=np.float32))
    W_proj = np.asarray(W_proj, dtype=np.float32)
    b_proj = np.asarray(b_proj, dtype=np.float32)
    W_out = np.asarray(W_out, dtype=np.float32)
    b_out = np.asarray(b_out, dtype=np.float32)

    x2 = x.reshape(N_CORES, C, SEQ)
    xa_all = np.empty((N_CORES, CA, SEQ), dtype=bf)
    xa_all[:, :C, :] = x2.astype(bf)
    xa_all[:, C, :] = np.float32(1.0)

    # XaT: [core][p, jt, c'] = x[c', jt*128+p], ones at c'=64
    xt_all = np.empty((N_CORES, 128, JT, CA), dtype=bf)
    xtt = x2.transpose(0, 2, 1).reshape(N_CORES, JT, 128, C)  # [b, jt, p, c]
    xt_all[:, :, :, :C] = xtt.transpose(0, 2, 1, 3).astype(bf)
    xt_all[:, :, :, C] = np.float32(1.0)

    # augmented per-head projection blocks [65, 640]
    Wa = np.concatenate([W_proj, b_proj[None, :]], axis=0)  # [65, 7680]
    gt = np.empty((CA, NH, CA), dtype=bf)
    ffm = np.empty((CA, NH, C), dtype=bf)
    for h in range(NH):
        q0 = h * 3 * D
        Wq = Wa[:, q0:q0 + D]            # [65, 640]
        Wk = Wa[:, q0 + D:q0 + 2 * D]
        Wv = Wa[:, q0 + 2 * D:q0 + 3 * D]
        G = Wk @ Wq.T                    # [65, 65]; scoresT = Xa^T G Xa
        gt[:, h, :] = G.T.astype(bf)     # lhsT[c', c] = G[c, c']
        F = Wv @ W_out[h * D:(h + 1) * D, :]   # [65, 64]
        ffm[:, h, :] = F.astype(bf)

    bo = np.ascontiguousarray(b_out.reshape(C, 1))

    return [
        {
            "xa": np.ascontiguousarray(xa_all[i]),
            "xt": np.ascontiguousarray(xt_all[i]),
            "xf": np.ascontiguousarray(x2[i]),
            "gt": gt,
            "ff": ffm,
            "bo": bo,
        }
        for i in range(N_CORES)
    ]


def run(x, t, W_proj, b_proj, W_out, b_out, trace=False, **trace_kwargs):
    in_maps = _prep_in_maps(x, W_proj, b_proj, W_out, b_out)
    res = run_bass_kernel_spmd(
        _get_nc(), in_maps, core_ids=list(range(N_CORES)),
        trace=trace, **trace_kwargs,
    )
    out = np.stack([res.results[i]["out"] for i in range(N_CORES)])
    return out.reshape(N_CORES, C, 32, 32), res


def kernel(x, t=None, W_proj=None, b_proj=None, W_out=None, b_out=None):
    out, _ = run(x, t, W_proj, b_proj, W_out, b_out, trace=False)
    return out


# revision 18
# speedup vs baseline: 9.3083x; 1.0019x over previous
"""Trainium2 Bass kernel for nn_AttLayer (4-head attention, softmax over queries).

Sharding: data-parallel over batch. 8 batch elements -> 8 NeuronCores, zero
collectives.

Key algebraic restructuring: with C=64 channels the attention is rank-65.
Folding the projections through the score/value contractions (bias rows
appended via the augmented-ones trick):

  R_h       = G_h^T-contracted input               G_h = Wk_aug_h @ Wq_aug_h^T
  scoresT_h = Xa^T R_h                             (= Xa^T G_h Xa, 65 x 65 G)
  es        = exp(SCALE * scoresT)                 row-sum den fused into the
                                                   exp activation (accum_out)
  xtr[j,c]  = XaT[j,c] / den[j]                    reciprocal folded into the
                                                   65-wide transposed input
  M2_h[c,i] = sum_j xtr[j,c] * es[j,i]             (65 x 1024)
  out2     += F_h^T @ M2_h                         F_h = Wv_aug_h @ Wout_h
  out       = out2 + b_out + x

G_h and F_h are computed on the host in f32 (exact). Everything on-chip is
bf16 matmuls with f32 PSUM accumulation; the exp/normalize core is the
critical path (ScalarEngine), so all other work is software-pipelined into
the per-j-tile chain steps of neighboring heads.
"""

import numpy as np
import ml_dtypes

import concourse.bass as bass
import concourse.tile as tile
from concourse import bacc, mybir
from concourse.bass_utils import run_bass_kernel_spmd

NH = 4          # heads
D = 640         # per-head dim
C = 64          # channels
CA = C + 1      # augmented (ones row)
SEQ = 1024      # 32*32
SCALE = float(D) ** -0.5
N_CORES = 8
FP = mybir.dt.float32
BF = mybir.dt.bfloat16

JT = SEQ // 128     # 8 j-tiles (128 keys each)
IC = SEQ // 512     # 2 i-chunks (512 queries each)

AF = mybir.ActivationFunctionType
ALU = mybir.AluOpType


def _build():
    nc = bacc.Bacc(None, target_bir_lowering=False)
    xa = nc.declare_dram_parameter("xa", [CA, SEQ], BF, isOutput=False)
    xt = nc.declare_dram_parameter("xt", [128, JT, CA], BF, isOutput=False)
    xf = nc.declare_dram_parameter("xf", [C, SEQ], FP, isOutput=False)
    gt = nc.declare_dram_parameter("gt", [CA, NH, CA], BF, isOutput=False)
    ff = nc.declare_dram_parameter("ff", [CA, NH, C], BF, isOutput=False)
    bo = nc.declare_dram_parameter("bo", [C, 1], FP, isOutput=False)
    out = nc.declare_dram_parameter("out", [C, SEQ], FP, isOutput=True)

    with tile.TileContext(nc) as tc:
        with (
            tc.tile_pool(name="consts", bufs=1) as consts,
            tc.tile_pool(name="hpool", bufs=2) as hpool,
            tc.tile_pool(name="sc", bufs=2, space="PSUM") as sc_psum,
            tc.tile_pool(name="pm", bufs=2, space="PSUM") as pm_psum,
        ):
            xa_sb = consts.tile([CA, SEQ], BF)
            for ic in range(IC):
                nc.sync.dma_start(
                    out=xa_sb[:, ic * 512:(ic + 1) * 512],
                    in_=xa[:, ic * 512:(ic + 1) * 512],
                )
            gt_sb = consts.tile([CA, NH, CA], BF)
            nc.sync.dma_start(out=gt_sb[:], in_=gt[:, :, :])
            xt_sb = consts.tile([128, JT, CA], BF)
            nc.sync.dma_start(out=xt_sb[:], in_=xt[:, :, :])
            ff_sb = consts.tile([CA, NH, C], BF)
            nc.sync.dma_start(out=ff_sb[:], in_=ff[:, :, :])
            xf_sb = consts.tile([C, SEQ], FP)
            nc.sync.dma_start(out=xf_sb[:], in_=xf[:, :])
            bo_sb = consts.tile([C, 1], FP)
            nc.sync.dma_start(out=bo_sb[:], in_=bo[:, :])
            out_sb = consts.tile([C, SEQ], FP)
            o2acc = consts.tile([C, SEQ], FP)

            def emit_R_ic(h, ic, state):
                if ic == 0:
                    state = (
                        hpool.tile([CA, SEQ], BF, tag="R", name=f"R_{h}"),
                        pm_psum.tile([CA, SEQ], FP, tag="pm", name=f"rp_{h}"),
                    )
                R_sb, rps = state
                nc.tensor.matmul(
                    rps[:, ic * 512:(ic + 1) * 512],
                    lhsT=gt_sb[:, h, :],
                    rhs=xa_sb[:, ic * 512:(ic + 1) * 512],
                    start=True, stop=True,
                )
                nc.vector.tensor_copy(
                    out=R_sb[:, ic * 512:(ic + 1) * 512],
                    in_=rps[:, ic * 512:(ic + 1) * 512],
                )
                return state

            def emit_R(h):
                state = emit_R_ic(h, 0, None)
                state = emit_R_ic(h, 1, state)
                return state[0]

            def emit_M2_mms(mps, xtr, es, jt):
                for ic in range(IC):
                    nc.tensor.matmul(
                        mps[:, ic * 512:(ic + 1) * 512],
                        lhsT=xtr[:, jt, :],
                        rhs=es[:, jt, ic * 512:(ic + 1) * 512],
                        start=(jt == 0), stop=(jt == JT - 1),
                    )

            def emit_m2_conv(ph, pmps):
                pm2 = hpool.tile([CA, SEQ], BF, tag="m2", name=f"m2_{ph}")
                for ic in range(IC):
                    nc.vector.tensor_copy(
                        out=pm2[:, ic * 512:(ic + 1) * 512],
                        in_=pmps[:, ic * 512:(ic + 1) * 512],
                    )
                return pm2

            def emit_out2(h, m2):
                o2p = pm_psum.tile([CA, SEQ], FP, tag="pm", name=f"o2_{h}")
                for ic in range(IC):
                    nc.tensor.matmul(
                        o2p[:C, ic * 512:(ic + 1) * 512],
                        lhsT=ff_sb[:, h, :],
                        rhs=m2[:, ic * 512:(ic + 1) * 512],
                        start=True, stop=True,
                    )
                if h == 0:
                    nc.vector.tensor_copy(out=o2acc[:], in_=o2p[:C, :])
                elif h < NH - 1:
                    nc.vector.tensor_add(out=o2acc[:], in0=o2acc[:], in1=o2p[:C, :])
                else:
                    # final head: o2acc already holds heads 0-2 plus residual
                    for ic in range(IC):
                        sl = slice(ic * 512, (ic + 1) * 512)
                        nc.scalar.activation(
                            out=out_sb[:, sl],
                            in_=o2p[:C, sl],
                            func=AF.Identity,
                            bias=bo_sb[:, 0:1],
                            scale=1.0,
                        )
                        nc.vector.tensor_add(
                            out=out_sb[:, sl], in0=out_sb[:, sl], in1=o2acc[:, sl],
                        )
                        for q in range(2):
                            qsl = slice(ic * 512 + q * 256, ic * 512 + (q + 1) * 256)
                            nc.sync.dma_start(out=out[:, qsl], in_=out_sb[:, qsl])

            R_cur = emit_R(0)
            R_nxt = None
            prev = None   # (h, es, xtr, mps) of the previous head
            for h in range(NH):
                R_sb = R_cur
                last = h == NH - 1
                es = hpool.tile([128, JT, SEQ], BF, tag="es", name=f"es_{h}")
                xtr = hpool.tile([128, JT, CA], BF, tag="xtr", name=f"xtr_{h}")
                den = hpool.tile([128, JT], FP, tag="den", name=f"den_{h}")
                rec = hpool.tile([128, JT], FP, tag="rec", name=f"rec_{h}")
                own_mps = (
                    pm_psum.tile([CA, SEQ], FP, tag="pm", name="mp_last")
                    if last else None
                )

                for jt in range(JT):
                    pst = sc_psum.tile([128, SEQ], FP, tag="sc", name=f"sc_{h}_{jt}")
                    for ic in range(IC):
                        nc.tensor.matmul(
                            pst[:, ic * 512:(ic + 1) * 512],
                            lhsT=xa_sb[:, jt * 128:(jt + 1) * 128],
                            rhs=R_sb[:, ic * 512:(ic + 1) * 512],
                            start=True, stop=True,
                        )
                    nc.scalar.activation(
                        out=es[:, jt, :],
                        in_=pst[:],
                        func=AF.Exp,
                        scale=SCALE,
                        accum_out=den[:, jt:jt + 1],
                    )
                    nc.vector.reciprocal(out=rec[:, jt:jt + 1], in_=den[:, jt:jt + 1])
                    nc.vector.tensor_scalar_mul(
                        xtr[:, jt, :], xt_sb[:, jt, :], rec[:, jt:jt + 1],
                    )

                    # ---- pipelined injections (<=2 matmuls per chain step)
                    if prev is not None:
                        ph, pes, pxtr, pmps = prev
                        emit_M2_mms(pmps, pxtr, pes, jt)
                        if jt == JT - 1:
                            pm2 = emit_m2_conv(ph, pmps)
                            emit_out2(ph, pm2)
                            prev = None
                    if jt == 2 and h + 1 < NH:
                        R_state = emit_R_ic(h + 1, 0, None)
                    if jt == 3 and h + 1 < NH:
                        R_nxt = emit_R_ic(h + 1, 1, R_state)[0]
                    if last and jt >= 1:
                        emit_M2_mms(own_mps, xtr, es, jt - 1)
                    if last and jt == 7:
                        nc.vector.tensor_add(
                            out=o2acc[:], in0=o2acc[:], in1=xf_sb[:],
                        )

                if not last:
                    mps = pm_psum.tile([CA, SEQ], FP, tag="pm", name=f"mp_{h}")
                    prev = (h, es, xtr, mps)
                R_cur = R_nxt

            # drain the last head's M2 tail (j-tile 7) and final output
            emit_M2_mms(own_mps, xtr, es, 7)
            pm2 = emit_m2_conv(NH - 1, own_mps)
            emit_out2(NH - 1, pm2)

    nc.compile()
    return nc


_CACHE: dict = {}


def _get_nc():
    if "nc" not in _CACHE:
        _CACHE["nc"] = _build()
    return _CACHE["nc"]


def _prep_in_maps(x, W_proj, b_proj, W_out, b_out):
    bf = ml_dtypes.bfloat16
    x = np.ascontiguousarray(np.asarray(x, dtype=np.float32))
    W_proj = np.asarray(W_proj, dtype=np.float32)
    b_proj = np.asarray(b_proj, dtype=np.float32)
    W_out = np.asarray(W_out, dtype=np.float32)
    b_out = np.asarray(b_out, dtype=np.float32)

    x2 = x.reshape(N_CORES, C, SEQ)
    xa_all = np.empty((N_CORES, CA, SEQ), dtype=bf)
    xa_all[:, :C, :] = x2.astype(bf)
    xa_all[:, C, :] = np.float32(1.0)

    # XaT: [core][p, jt, c'] = x[c', jt*128+p], ones at c'=64
    xt_all = np.empty((N_CORES, 128, JT, CA), dtype=bf)
    xtt = x2.transpose(0, 2, 1).reshape(N_CORES, JT, 128, C)  # [b, jt, p, c]
    xt_all[:, :, :, :C] = xtt.transpose(0, 2, 1, 3).astype(bf)
    xt_all[:, :, :, C] = np.float32(1.0)

    # augmented per-head projection blocks [65, 640]
    Wa = np.concatenate([W_proj, b_proj[None, :]], axis=0)  # [65, 7680]
    gt = np.empty((CA, NH, CA), dtype=bf)
    ffm = np.empty((CA, NH, C), dtype=bf)
    for h in range(NH):
        q0 = h * 3 * D
        Wq = Wa[:, q0:q0 + D]            # [65, 640]
        Wk = Wa[:, q0 + D:q0 + 2 * D]
        Wv = Wa[:, q0 + 2 * D:q0 + 3 * D]
        G = Wk @ Wq.T                    # [65, 65]; scoresT = Xa^T G Xa
        gt[:, h, :] = G.T.astype(bf)     # lhsT[c', c] = G[c, c']
        F = Wv @ W_out[h * D:(h + 1) * D, :]   # [65, 64]
        ffm[:, h, :] = F.astype(bf)

    bo = np.ascontiguousarray(b_out.reshape(C, 1))

    return [
        {
            "xa": np.ascontiguousarray(xa_all[i]),
            "xt": np.ascontiguousarray(xt_all[i]),
            "xf": np.ascontiguousarray(x2[i]),
            "gt": gt,
            "ff": ffm,
            "bo": bo,
        }
        for i in range(N_CORES)
    ]


def run(x, t, W_proj, b_proj, W_out, b_out, trace=False, **trace_kwargs):
    in_maps = _prep_in_maps(x, W_proj, b_proj, W_out, b_out)
    res = run_bass_kernel_spmd(
        _get_nc(), in_maps, core_ids=list(range(N_CORES)),
        trace=trace, **trace_kwargs,
    )
    out = np.stack([res.results[i]["out"] for i in range(N_CORES)])
    return out.reshape(N_CORES, C, 32, 32), res


def kernel(x, t=None, W_proj=None, b_proj=None, W_out=None, b_out=None):
    out, _ = run(x, t, W_proj, b_proj, W_out, b_out, trace=False)
    return out


# revision 19
# speedup vs baseline: 9.3389x; 1.0033x over previous
"""Trainium2 Bass kernel for nn_AttLayer (4-head attention, softmax over queries).

Sharding: data-parallel over batch. 8 batch elements -> 8 NeuronCores, zero
collectives.

Key algebraic restructuring: with C=64 channels the attention is rank-65.
Folding the projections through the score/value contractions (bias rows
appended via the augmented-ones trick):

  R_h       = G_h^T-contracted input               G_h = Wk_aug_h @ Wq_aug_h^T
  scoresT_h = Xa^T R_h                             (= Xa^T G_h Xa, 65 x 65 G)
  es        = exp(SCALE * scoresT)                 row-sum den fused into the
                                                   exp activation (accum_out)
  xtr[j,c]  = XaT[j,c] / den[j]                    reciprocal folded into the
                                                   65-wide transposed input
  M2_h[c,i] = sum_j xtr[j,c] * es[j,i]             (65 x 1024)
  out2     += F_h^T @ M2_h                         F_h = Wv_aug_h @ Wout_h
  out       = out2 + b_out + x

G_h and F_h are computed on the host in f32 (exact). Everything on-chip is
bf16 matmuls with f32 PSUM accumulation; the exp/normalize core is the
critical path (ScalarEngine), so all other work is software-pipelined into
the per-j-tile chain steps of neighboring heads.
"""

import numpy as np
import ml_dtypes

import concourse.bass as bass
import concourse.tile as tile
from concourse import bacc, mybir
from concourse.bass_utils import run_bass_kernel_spmd

NH = 4          # heads
D = 640         # per-head dim
C = 64          # channels
CA = C + 1      # augmented (ones row)
SEQ = 1024      # 32*32
SCALE = float(D) ** -0.5
N_CORES = 8
FP = mybir.dt.float32
BF = mybir.dt.bfloat16

JT = SEQ // 128     # 8 j-tiles (128 keys each)
IC = SEQ // 512     # 2 i-chunks (512 queries each)

AF = mybir.ActivationFunctionType
ALU = mybir.AluOpType


def _build():
    nc = bacc.Bacc(None, target_bir_lowering=False)
    xa = nc.declare_dram_parameter("xa", [CA, SEQ], BF, isOutput=False)
    xt = nc.declare_dram_parameter("xt", [128, JT, CA], BF, isOutput=False)
    xf = nc.declare_dram_parameter("xf", [C, SEQ], FP, isOutput=False)
    gt = nc.declare_dram_parameter("gt", [CA, NH, CA], BF, isOutput=False)
    ff = nc.declare_dram_parameter("ff", [CA, NH, C], BF, isOutput=False)
    bo = nc.declare_dram_parameter("bo", [C, 1], FP, isOutput=False)
    out = nc.declare_dram_parameter("out", [C, SEQ], FP, isOutput=True)

    with tile.TileContext(nc) as tc:
        with (
            tc.tile_pool(name="consts", bufs=1) as consts,
            tc.tile_pool(name="hpool", bufs=3) as hpool,
            tc.tile_pool(name="sc", bufs=2, space="PSUM") as sc_psum,
            tc.tile_pool(name="pm", bufs=2, space="PSUM") as pm_psum,
        ):
            xa_sb = consts.tile([CA, SEQ], BF)
            for ic in range(IC):
                nc.sync.dma_start(
                    out=xa_sb[:, ic * 512:(ic + 1) * 512],
                    in_=xa[:, ic * 512:(ic + 1) * 512],
                )
            gt_sb = consts.tile([CA, NH, CA], BF)
            nc.sync.dma_start(out=gt_sb[:], in_=gt[:, :, :])
            xt_sb = consts.tile([128, JT, CA], BF)
            nc.sync.dma_start(out=xt_sb[:], in_=xt[:, :, :])
            ff_sb = consts.tile([CA, NH, C], BF)
            nc.sync.dma_start(out=ff_sb[:], in_=ff[:, :, :])
            xf_sb = consts.tile([C, SEQ], FP)
            nc.sync.dma_start(out=xf_sb[:], in_=xf[:, :])
            bo_sb = consts.tile([C, 1], FP)
            nc.sync.dma_start(out=bo_sb[:], in_=bo[:, :])
            out_sb = consts.tile([C, SEQ], FP)
            o2acc = consts.tile([C, SEQ], FP)

            def emit_R_ic(h, ic, state):
                if ic == 0:
                    state = (
                        hpool.tile([CA, SEQ], BF, tag="R", name=f"R_{h}"),
                        pm_psum.tile([CA, SEQ], FP, tag="pm", name=f"rp_{h}"),
                    )
                R_sb, rps = state
                nc.tensor.matmul(
                    rps[:, ic * 512:(ic + 1) * 512],
                    lhsT=gt_sb[:, h, :],
                    rhs=xa_sb[:, ic * 512:(ic + 1) * 512],
                    start=True, stop=True,
                )
                nc.vector.tensor_copy(
                    out=R_sb[:, ic * 512:(ic + 1) * 512],
                    in_=rps[:, ic * 512:(ic + 1) * 512],
                )
                return state

            def emit_R(h):
                state = emit_R_ic(h, 0, None)
                state = emit_R_ic(h, 1, state)
                return state[0]

            def emit_M2_mms(mps, xtr, es, jt):
                for ic in range(IC):
                    nc.tensor.matmul(
                        mps[:, ic * 512:(ic + 1) * 512],
                        lhsT=xtr[:, jt, :],
                        rhs=es[:, jt, ic * 512:(ic + 1) * 512],
                        start=(jt == 0), stop=(jt == JT - 1),
                    )

            def emit_m2_conv(ph, pmps):
                pm2 = hpool.tile([CA, SEQ], BF, tag="m2", name=f"m2_{ph}")
                for ic in range(IC):
                    nc.vector.tensor_copy(
                        out=pm2[:, ic * 512:(ic + 1) * 512],
                        in_=pmps[:, ic * 512:(ic + 1) * 512],
                    )
                return pm2

            def emit_out2(h, m2):
                o2p = pm_psum.tile([CA, SEQ], FP, tag="pm", name=f"o2_{h}")
                for ic in range(IC):
                    nc.tensor.matmul(
                        o2p[:C, ic * 512:(ic + 1) * 512],
                        lhsT=ff_sb[:, h, :],
                        rhs=m2[:, ic * 512:(ic + 1) * 512],
                        start=True, stop=True,
                    )
                if h == 0:
                    nc.vector.tensor_copy(out=o2acc[:], in_=o2p[:C, :])
                elif h < NH - 1:
                    nc.vector.tensor_add(out=o2acc[:], in0=o2acc[:], in1=o2p[:C, :])
                else:
                    # final head: o2acc already holds heads 0-2 plus residual
                    for ic in range(IC):
                        sl = slice(ic * 512, (ic + 1) * 512)
                        nc.scalar.activation(
                            out=out_sb[:, sl],
                            in_=o2p[:C, sl],
                            func=AF.Identity,
                            bias=bo_sb[:, 0:1],
                            scale=1.0,
                        )
                        nc.vector.tensor_add(
                            out=out_sb[:, sl], in0=out_sb[:, sl], in1=o2acc[:, sl],
                        )
                        for q in range(2):
                            qsl = slice(ic * 512 + q * 256, ic * 512 + (q + 1) * 256)
                            nc.sync.dma_start(out=out[:, qsl], in_=out_sb[:, qsl])

            R_cur = emit_R(0)
            R_nxt = None
            prev = None   # (h, es, xtr, mps) of the previous head
            for h in range(NH):
                R_sb = R_cur
                last = h == NH - 1
                es = hpool.tile([128, JT, SEQ], BF, tag="es", name=f"es_{h}")
                xtr = hpool.tile([128, JT, CA], BF, tag="xtr", name=f"xtr_{h}")
                den = hpool.tile([128, JT], FP, tag="den", name=f"den_{h}")
                rec = hpool.tile([128, JT], FP, tag="rec", name=f"rec_{h}")
                own_mps = (
                    pm_psum.tile([CA, SEQ], FP, tag="pm", name="mp_last")
                    if last else None
                )

                for jt in range(JT):
                    pst = sc_psum.tile([128, SEQ], FP, tag="sc", name=f"sc_{h}_{jt}")
                    for ic in range(IC):
                        nc.tensor.matmul(
                            pst[:, ic * 512:(ic + 1) * 512],
                            lhsT=xa_sb[:, jt * 128:(jt + 1) * 128],
                            rhs=R_sb[:, ic * 512:(ic + 1) * 512],
                            start=True, stop=True,
                        )
                    nc.scalar.activation(
                        out=es[:, jt, :],
                        in_=pst[:],
                        func=AF.Exp,
                        scale=SCALE,
                        accum_out=den[:, jt:jt + 1],
                    )
                    nc.vector.reciprocal(out=rec[:, jt:jt + 1], in_=den[:, jt:jt + 1])
                    nc.vector.tensor_scalar_mul(
                        xtr[:, jt, :], xt_sb[:, jt, :], rec[:, jt:jt + 1],
                    )

                    # ---- pipelined injections (<=2 matmuls per chain step)
                    if prev is not None:
                        ph, pes, pxtr, pmps = prev
                        emit_M2_mms(pmps, pxtr, pes, jt)
                        if jt == JT - 1:
                            pm2 = emit_m2_conv(ph, pmps)
                            emit_out2(ph, pm2)
                            prev = None
                    if jt == 2 and h + 1 < NH:
                        R_state = emit_R_ic(h + 1, 0, None)
                    if jt == 3 and h + 1 < NH:
                        R_nxt = emit_R_ic(h + 1, 1, R_state)[0]
                    if last and jt >= 1:
                        emit_M2_mms(own_mps, xtr, es, jt - 1)
                    if last and jt == 7:
                        nc.vector.tensor_add(
                            out=o2acc[:], in0=o2acc[:], in1=xf_sb[:],
                        )

                if not last:
                    mps = pm_psum.tile([CA, SEQ], FP, tag="pm", name=f"mp_{h}")
                    prev = (h, es, xtr, mps)
                R_cur = R_nxt

            # drain the last head's M2 tail (j-tile 7) and final output
            emit_M2_mms(own_mps, xtr, es, 7)
            pm2 = emit_m2_conv(NH - 1, own_mps)
            emit_out2(NH - 1, pm2)

    nc.compile()
    return nc


_CACHE: dict = {}


def _get_nc():
    if "nc" not in _CACHE:
        _CACHE["nc"] = _build()
    return _CACHE["nc"]


def _prep_in_maps(x, W_proj, b_proj, W_out, b_out):
    bf = ml_dtypes.bfloat16
    x = np.ascontiguousarray(np.asarray(x, dtype=np.float32))
    W_proj = np.asarray(W_proj, dtype=np.float32)
    b_proj = np.asarray(b_proj, dtype=np.float32)
    W_out = np.asarray(W_out, dtype=np.float32)
    b_out = np.asarray(b_out, dtype=np.float32)

    x2 = x.reshape(N_CORES, C, SEQ)
    xa_all = np.empty((N_CORES, CA, SEQ), dtype=bf)
    xa_all[:, :C, :] = x2.astype(bf)
    xa_all[:, C, :] = np.float32(1.0)

    # XaT: [core][p, jt, c'] = x[c', jt*128+p], ones at c'=64
    xt_all = np.empty((N_CORES, 128, JT, CA), dtype=bf)
    xtt = x2.transpose(0, 2, 1).reshape(N_CORES, JT, 128, C)  # [b, jt, p, c]
    xt_all[:, :, :, :C] = xtt.transpose(0, 2, 1, 3).astype(bf)
    xt_all[:, :, :, C] = np.float32(1.0)

    # augmented per-head projection blocks [65, 640]
    Wa = np.concatenate([W_proj, b_proj[None, :]], axis=0)  # [65, 7680]
    gt = np.empty((CA, NH, CA), dtype=bf)
    ffm = np.empty((CA, NH, C), dtype=bf)
    for h in range(NH):
        q0 = h * 3 * D
        Wq = Wa[:, q0:q0 + D]            # [65, 640]
        Wk = Wa[:, q0 + D:q0 + 2 * D]
        Wv = Wa[:, q0 + 2 * D:q0 + 3 * D]
        G = Wk @ Wq.T                    # [65, 65]; scoresT = Xa^T G Xa
        gt[:, h, :] = G.T.astype(bf)     # lhsT[c', c] = G[c, c']
        F = Wv @ W_out[h * D:(h + 1) * D, :]   # [65, 64]
        ffm[:, h, :] = F.astype(bf)

    bo = np.ascontiguousarray(b_out.reshape(C, 1))

    return [
        {
            "xa": np.ascontiguousarray(xa_all[i]),
            "xt": np.ascontiguousarray(xt_all[i]),
            "xf": np.ascontiguousarray(x2[i]),
            "gt": gt,
            "ff": ffm,
            "bo": bo,
        }
        for i in range(N_CORES)
    ]


def run(x, t, W_proj, b_proj, W_out, b_out, trace=False, **trace_kwargs):
    in_maps = _prep_in_maps(x, W_proj, b_proj, W_out, b_out)
    res = run_bass_kernel_spmd(
        _get_nc(), in_maps, core_ids=list(range(N_CORES)),
        trace=trace, **trace_kwargs,
    )
    out = np.stack([res.results[i]["out"] for i in range(N_CORES)])
    return out.reshape(N_CORES, C, 32, 32), res


def kernel(x, t=None, W_proj=None, b_proj=None, W_out=None, b_out=None):
    out, _ = run(x, t, W_proj, b_proj, W_out, b_out, trace=False)
    return out
